# revision 43
# baseline (speedup 1.0000x reference)
"""AttentionPairBias kernel for Trainium2, 8-core SPMD.

Problem (reference.py):
  q = (s @ Wq + bq); k = k_in @ Wk; v = k_in @ Wv       (B,N,H,D)
  zn = LayerNorm(z) * ln_g + ln_b                        (B,N,N,CZ)
  bias = transpose(zn @ Wz) -> (B,H,N,N)
  g = sigmoid(s @ Wg)
  p = softmax_j(q.k/sqrt(D) + bias + maskterm)
  out = (g * (p @ v)) @ Wo

Sharding: 8 cores = (batch b in {0,1}) x (4 slices of 192 query rows i).
Each core computes out[b, i0:i0+192, :] completely (full key range
on-core), so the host only slices inputs / concatenates outputs.
Host-side prep is layout-only: slicing, transposes (sT, kinT, per-row zT).

Device math restructuring:
  LayerNorm folded into the Wz projection:
    bias[pair,h] = rstd[pair] * (z[pair,:] @ W''[:,h])  (+ per-(b,h,i) const
    dropped -- softmax over j is invariant to it; this absorbs ln_b and the
    mean term)
  W'[c,h] = ln_g[c]*Wz[c,h];  W''[c,h] = W'[c,h] - mean_c W'[:,h]
  rstd from per-pair sum(z) (ones-column riding the same matmul) and
  sum(z^2) (ones-matmul over squared z).
"""

import os
import numpy as np

B, N, CS, CZ, H, D = 2, 768, 1024, 128, 16, 64
NCORES = 8
IPC = N // 4            # 192 query rows per core
CT_host = CS // 128
LN_EPS = 1e-5
INV_SQRT_D = 0.125
INF = 1000000.0

_prog_cache = {}


def _build(ipc=IPC, mask_ones=True, repeat=1):
    import contextlib
    import concourse.bass as bass
    import concourse.tile as tile
    from concourse import bacc, mybir

    f32 = mybir.dt.float32
    bf16 = mybir.dt.bfloat16
    AF = mybir.ActivationFunctionType
    OP = mybir.AluOpType

    CT = CS // 128          # 8 c_s tiles
    JT = N // 128           # 6 j tiles
    assert ipc % 4 == 0
    # 3 x 64-row i-tiles alternating PH partition halves (poff 0/64/0):
    # tile t+1's z-stream writes PH partitions disjoint from tile t's
    # attention reads, so the phases can overlap instead of serializing
    # on the PH_nat WAR hazard.
    itiles = []
    o = 0
    t = 0
    while o < ipc:
        isz = min(64, ipc - o)
        itiles.append((o, isz, 64 * (t % 2)))
        o += isz
        t += 1

    nc = bacc.Bacc("TRN2", target_bir_lowering=False, debug=False,
                   enable_asserts=False, num_devices=NCORES)

    # ---- DRAM I/O ----
    # z/s/k_in/W* ship as bf16 (host cast): halves HBM traffic; compute
    # was bf16 in SBUF already.
    zT_d = nc.dram_tensor("zT", [ipc * CZ, N], bf16, kind="ExternalInput")
    sT_d = nc.dram_tensor("sT", [CS, ipc], bf16, kind="ExternalInput")
    kinT_d = nc.dram_tensor("kinT", [CS, N], bf16, kind="ExternalInput")
    W_d = {w: nc.dram_tensor(w, [CS, CS], bf16, kind="ExternalInput")
           for w in ("Wq", "Wk", "Wv", "Wg", "Wo")}
    Wz_d = nc.dram_tensor("Wz", [CZ, H], f32, kind="ExternalInput")
    lng_d = nc.dram_tensor("ln_g", [CZ, 1], f32, kind="ExternalInput")
    bqT_d = nc.dram_tensor("bqT", [128, CT], f32, kind="ExternalInput")
    if not mask_ones:
        mask_d = nc.dram_tensor("maskrow", [1, N], f32, kind="ExternalInput")
    outT_d = nc.dram_tensor("outT", [CS, ipc], f32, kind="ExternalOutput")

    with tile.TileContext(nc) as tc:
        # ---------------- persistent SBUF ----------------
        Wsb = {w: nc.alloc_sbuf_tensor(f"{w}_sb", [128, CT, CS], bf16)
               for w in ("Wq", "Wk", "Wv", "Wg", "Wo")}
        kinT_sb = nc.alloc_sbuf_tensor("kinT_sb", [128, CT, N], bf16)
        sT_sb = nc.alloc_sbuf_tensor("sT_sb", [128, CT, ipc], bf16)
        qT_sb = nc.alloc_sbuf_tensor("qT_sb", [128, CT, ipc], bf16)
        kT_sb = nc.alloc_sbuf_tensor("kT_sb", [128, CT, N], bf16)
        v_sb = nc.alloc_sbuf_tensor("v_sb", [128, JT, CS], bf16)
        gT_sb = nc.alloc_sbuf_tensor("gT_sb", [128, CT, ipc], bf16)
        goT_sb = nc.alloc_sbuf_tensor("goT_sb", [128, CT, 128], bf16)
        bqT_sb = nc.alloc_sbuf_tensor("bqT_sb", [128, CT], f32)
        Wz_sb = nc.alloc_sbuf_tensor("Wz_sb", [128, H], f32)
        lng_sb = nc.alloc_sbuf_tensor("lng_sb", [128, 1], f32)
        Wp_sb = nc.alloc_sbuf_tensor("Wp_sb", [128, H], f32)
        G_sb = nc.alloc_sbuf_tensor("G_sb", [128, H], f32)
        Grow_sb = nc.alloc_sbuf_tensor("Grow_sb", [1, H], f32)
        Waug = nc.alloc_sbuf_tensor("Waug", [128, 32], bf16)
        ones17 = nc.alloc_sbuf_tensor("ones17", [128, 32], bf16)
        id_sb = nc.alloc_sbuf_tensor("id_sb", [128, 128], bf16)
        eps_sb = nc.alloc_sbuf_tensor("eps_sb", [128, 1], f32)
        # per-i-tile working buffers: rows 0..15 = P, 16 = sum z, 17 = sum z^2
        PH_nat = nc.alloc_sbuf_tensor("PH_nat", [128, H + 2, N], bf16)
        stat_a = nc.alloc_sbuf_tensor("stat_a", [128, N], f32)
        stat_b = nc.alloc_sbuf_tensor("stat_b", [128, N], f32)
        alpha = nc.alloc_sbuf_tensor("alpha", [128, N], bf16)
        den_sb = nc.alloc_sbuf_tensor("den_sb", [128, 2 * H], f32)
        rden_sb = nc.alloc_sbuf_tensor("rden_sb", [128, 2 * H], f32)
        if not mask_ones:
            mrow_sb = nc.alloc_sbuf_tensor("mrow_sb", [1, N], f32)
            mbias_sb = nc.alloc_sbuf_tensor("mbias_sb", [1, N], f32)
            mb_full = nc.alloc_sbuf_tensor("mb_full", [128, N], bf16)

        ctx = contextlib.ExitStack()
        with ctx:
            ps = ctx.enter_context(tc.tile_pool(name="ps", bufs=8, space="PSUM"))
            zpool = ctx.enter_context(tc.tile_pool(name="zs", bufs=4))
            tpool = ctx.enter_context(tc.tile_pool(name="ts", bufs=2))

            # ---------------- constant / weight prep ----------------
            nc.sync.dma_start(out=bqT_sb[:, :], in_=bqT_d.ap())
            nc.sync.dma_start(out=Wz_sb[:, :], in_=Wz_d.ap())
            nc.sync.dma_start(out=lng_sb[:, :], in_=lng_d.ap())
            nc.vector.memset(eps_sb[:, :], LN_EPS)
            # ones17: one-hot ones at col 17 -> S matmul accumulates sum(z^2)
            # into row 17 of the P matmul's PSUM tile (rows 0..16 += 0)
            nc.vector.memset(ones17[:, :], 0.0)
            nc.vector.memset(ones17[:, 17:18], 1.0)
            nc.vector.memset(Waug[:, :], 0.0)
            from concourse.masks import make_identity
            make_identity(nc, id_sb[:, :])
            # W' = ln_g*Wz ; G = colsum(W') ; W'' = W' - G/128
            nc.vector.tensor_scalar_mul(out=Wp_sb[:, :], in0=Wz_sb[:, :],
                                        scalar1=lng_sb[:, :])
            nc.vector.tensor_copy(out=Waug[:, 0:H], in_=Wp_sb[:, :])
            nc.vector.memset(Waug[:, H:H + 1], 1.0)
            G_ps = ps.tile([1, H], f32, tag="bank")
            nc.tensor.matmul(out=G_ps[:, :], lhsT=Waug[:, H:H + 1],
                             rhs=Waug[:, 0:H], start=True, stop=True)
            nc.vector.tensor_copy(out=Grow_sb[:, :], in_=G_ps[:, :])
            Grow_dram = nc.dram_tensor("Grow_dram", [H], f32, kind="Internal")
            nc.sync.dma_start(out=Grow_dram.ap(), in_=Grow_sb[:, :])
            nc.sync.dma_start(
                out=G_sb[:, :],
                in_=bass.AP(tensor=Grow_dram, offset=0, ap=[[0, 128], [1, H]]))
            nc.vector.scalar_tensor_tensor(out=Wp_sb[:, :], in0=G_sb[:, :],
                                           scalar=-1.0 / 128.0, in1=Wp_sb[:, :],
                                           op0=OP.mult, op1=OP.add)
            nc.vector.tensor_copy(out=Waug[:, 0:H], in_=Wp_sb[:, :])
            if not mask_ones:
                nc.sync.dma_start(out=mrow_sb[:, :], in_=mask_d.ap())
                nc.vector.tensor_scalar(out=mbias_sb[:, :], in0=mrow_sb[:, :],
                                        scalar1=1.0, scalar2=INF,
                                        op0=OP.subtract, op1=OP.mult)
                mb_dram = nc.dram_tensor("mb_dram", [N], f32, kind="Internal")
                nc.sync.dma_start(out=mb_dram.ap(), in_=mbias_sb[:, :])
                nc.gpsimd.dma_start(
                    out=mb_full[:, :],
                    in_=bass.AP(tensor=mb_dram, offset=0, ap=[[0, 128], [1, N]]))

            # weight / activation loads (bf16 in DRAM already)
            for w in ("Wq", "Wk", "Wv", "Wg", "Wo"):
                nc.gpsimd.dma_start(
                    out=Wsb[w][:, :, :],
                    in_=W_d[w].ap().rearrange("(t p) f -> p t f", p=128))
            nc.gpsimd.dma_start(
                out=kinT_sb[:, :, :],
                in_=kinT_d.ap().rearrange("(t p) j -> p t j", p=128))
            nc.gpsimd.dma_start(
                out=sT_sb[:, :, :],
                in_=sT_d.ap().rearrange("(t p) i -> p t i", p=128))

            for _rep in range(repeat):
                # ---------------- projections ----------------
                for f in range(CT):
                    g_ps = ps.tile([128, ipc], f32, tag="bank")
                    for c in range(CT):
                        nc.tensor.matmul(out=g_ps[:, :],
                                         lhsT=Wsb["Wg"][:, c, 128 * f:128 * (f + 1)],
                                         rhs=sT_sb[:, c, :],
                                         start=(c == 0), stop=(c == CT - 1))
                    nc.scalar.activation(out=gT_sb[:, f, :], in_=g_ps[:, :], func=AF.Sigmoid)
                for f in range(CT):
                    q_ps = ps.tile([128, ipc], f32, tag="bank")
                    for c in range(CT):
                        nc.tensor.matmul(out=q_ps[:, :],
                                         lhsT=Wsb["Wq"][:, c, 128 * f:128 * (f + 1)],
                                         rhs=sT_sb[:, c, :],
                                         start=(c == 0), stop=(c == CT - 1))
                    nc.vector.tensor_scalar_add(out=qT_sb[:, f, :], in0=q_ps[:, :],
                                                scalar1=bqT_sb[:, f:f + 1])
                for f in range(CT):
                    for hf in range(2):
                        k_ps = ps.tile([128, 384], f32, tag="bank")
                        for c in range(CT):
                            nc.tensor.matmul(out=k_ps[:, :],
                                             lhsT=Wsb["Wk"][:, c, 128 * f:128 * (f + 1)],
                                             rhs=kinT_sb[:, c, 384 * hf:384 * (hf + 1)],
                                             start=(c == 0), stop=(c == CT - 1))
                        nc.vector.tensor_copy(out=kT_sb[:, f, 384 * hf:384 * (hf + 1)],
                                              in_=k_ps[:, :])
                for jt in range(JT):
                    for hf in range(2):
                        v_ps = ps.tile([128, 512], f32, tag="bank")
                        for c in range(CT):
                            nc.tensor.matmul(out=v_ps[:, :],
                                             lhsT=kinT_sb[:, c, 128 * jt:128 * (jt + 1)],
                                             rhs=Wsb["Wv"][:, c, 512 * hf:512 * (hf + 1)],
                                             start=(c == 0), stop=(c == CT - 1))
                        nc.vector.tensor_copy(out=v_sb[:, jt, 512 * hf:512 * (hf + 1)],
                                              in_=v_ps[:, :])

                # ---------------- main loop over i-tiles ----------------
                for (i0, isz, poff) in itiles:
                    # ---- z stream: 4 i-rows per DMA (host-pretransposed bf16 zT)
                    # on the gpsimd SWDGE ring, decoupled from the small
                    # HWDGE transpose DMAs ----
                    for r4 in range(i0, i0 + isz, 4):
                        zTt = zpool.tile([128, 4, N], bf16, tag="zTt")
                        zsrc = bass.AP(
                            tensor=zT_d,
                            offset=r4 * CZ * N,
                            ap=[[N, 128], [CZ * N, 4], [1, N]],
                        )
                        eng_z = nc.gpsimd if (r4 // 4) % 2 == 0 else nc.sync
                        eng_z.dma_start(out=zTt[:, :, :], in_=zsrc)
                        Ppk = [ps.tile([128, 384], f32, tag="bank", name=f"Ppk{_h}")
                               for _h in range(2)]
                        for rl in range(4):
                            zsqT = zpool.tile([128, N], bf16, tag="zsqT")
                            if rl % 2 == 0:
                                nc.scalar.activation(out=zsqT[:, :], in_=zTt[:, rl, :],
                                                     func=AF.Square)
                            else:
                                nc.vector.scalar_tensor_tensor(
                                    out=zsqT[:, :], in0=zTt[:, rl, :], scalar=1.0,
                                    in1=zTt[:, rl, :], op0=OP.mult, op1=OP.mult)
                            for hf in range(2):
                                nc.tensor.matmul(out=Ppk[hf][32 * rl:32 * rl + 18, :],
                                                 lhsT=Waug[:, 0:18],
                                                 rhs=zTt[:, rl, 384 * hf:384 * (hf + 1)],
                                                 start=True, stop=False,
                                                 tile_position=(0, 32 * rl))
                                nc.tensor.matmul(out=Ppk[hf][32 * rl:32 * rl + 18, :],
                                                 lhsT=ones17[:, 0:18],
                                                 rhs=zsqT[:, 384 * hf:384 * (hf + 1)],
                                                 start=False, stop=True,
                                                 tile_position=(0, 32 * rl))
                        PSsb = tpool.tile([128, 2, 384], bf16, tag="PSsb")
                        for hf in range(2):
                            nc.vector.tensor_copy(out=PSsb[:, hf, :],
                                                  in_=Ppk[hf][:, :])
                        for rl in range(4):
                            li = r4 + rl - i0 + poff
                            eng_p = nc.sync if rl % 2 == 0 else nc.scalar
                            eng_p.dma_start(out=PH_nat[li:li + 1, :, :],
                                            in_=PSsb[32 * rl:32 * rl + H + 2, :, :])

                    # ---- stats -> alpha ; bias = alpha * P (in place) ----
                    sl = slice(poff, poff + isz)
                    nc.vector.scalar_tensor_tensor(
                        out=stat_a[sl, :], in0=PH_nat[sl, H, :],
                        scalar=1.0 / (128.0 * 128.0), in1=PH_nat[sl, H, :],
                        op0=OP.mult, op1=OP.mult)
                    nc.vector.scalar_tensor_tensor(
                        out=stat_b[sl, :], in0=PH_nat[sl, H + 1, :], scalar=1.0 / 128.0,
                        in1=stat_a[sl, :], op0=OP.mult, op1=OP.subtract)
                    # rstd = exp(-0.5*ln(var+eps)) -- keeps ACT on one table set
                    nc.scalar.activation(out=stat_b[sl, :], in_=stat_b[sl, :],
                                         func=AF.Ln, bias=eps_sb[sl, :], scale=1.0)
                    nc.scalar.activation(out=alpha[sl, :], in_=stat_b[sl, :],
                                         func=AF.Exp, scale=-0.5)

                    # ---- attention ----
                    for h in range(H):
                        hp, off = h // 2, 64 * (h % 2)
                        nc.vector.scalar_tensor_tensor(
                            out=PH_nat[sl, h, :], in0=PH_nat[sl, h, :], scalar=1.0,
                            in1=alpha[sl, :], op0=OP.mult, op1=OP.mult)
                        if not mask_ones:
                            nc.vector.tensor_add(out=PH_nat[sl, h, :],
                                                 in0=PH_nat[sl, h, :],
                                                 in1=mb_full[sl, :])
                        qk0 = ps.tile([128, 384], f32, tag="bank")
                        qk1 = ps.tile([128, 384], f32, tag="bank")
                        p_sb = tpool.tile([128, N], bf16, tag="p_sb")
                        for hf, qk in ((0, qk0), (1, qk1)):
                            nc.tensor.matmul(out=qk[sl, :],
                                             lhsT=qT_sb[off:off + 64, hp, i0:i0 + isz],
                                             rhs=kT_sb[off:off + 64, hp, 384 * hf:384 * (hf + 1)],
                                             start=True, stop=True)
                            nc.vector.scalar_tensor_tensor(
                                out=qk[sl, :], in0=qk[sl, :], scalar=INV_SQRT_D,
                                in1=PH_nat[sl, h, 384 * hf:384 * (hf + 1)],
                                op0=OP.mult, op1=OP.add)
                            nc.scalar.activation(out=p_sb[sl, 384 * hf:384 * (hf + 1)],
                                                 in_=qk[sl, :], func=AF.Exp,
                                                 accum_out=den_sb[sl, 2 * h + hf:2 * h + hf + 1])
                        nc.vector.tensor_add(out=den_sb[sl, 2 * h:2 * h + 1],
                                             in0=den_sb[sl, 2 * h:2 * h + 1],
                                             in1=den_sb[sl, 2 * h + 1:2 * h + 2])
                        nc.vector.reciprocal(out=rden_sb[sl, h:h + 1],
                                             in_=den_sb[sl, 2 * h:2 * h + 1])
                        nc.vector.tensor_scalar_mul(out=p_sb[sl, :], in0=p_sb[sl, :],
                                                    scalar1=rden_sb[sl, h:h + 1])
                        pT_ps = ps.tile([128, JT, 128], bf16, tag="bank")
                        for jt in range(JT):
                            nc.tensor.transpose(out=pT_ps[:, jt, :isz],
                                                in_=p_sb[sl, 128 * jt:128 * (jt + 1)],
                                                identity=id_sb[sl, sl])
                        pT = tpool.tile([128, JT, 128], bf16, tag="pT")
                        nc.vector.tensor_copy(out=pT[:, :, :isz], in_=pT_ps[:, :, :isz])
                        if h % 2 == 0:
                            oT_ps = ps.tile([128, 128], f32, tag="bank")
                        for jt in range(JT):
                            nc.tensor.matmul(out=oT_ps[off:off + 64, :isz],
                                             lhsT=v_sb[:, jt, 64 * h:64 * (h + 1)],
                                             rhs=pT[:, jt, :isz],
                                             start=(jt == 0), stop=(jt == JT - 1))
                        if h % 2 == 1:
                            nc.vector.tensor_mul(out=goT_sb[:, hp, poff:poff + isz],
                                                 in0=oT_ps[:, :isz],
                                                 in1=gT_sb[:, hp, i0:i0 + isz])

                    # ---- output projection ----
                    for f in range(CT):
                        o_ps = ps.tile([128, 128], f32, tag="bank")
                        for c in range(CT):
                            nc.tensor.matmul(out=o_ps[:, :isz],
                                             lhsT=Wsb["Wo"][:, c, 128 * f:128 * (f + 1)],
                                             rhs=goT_sb[:, c, poff:poff + isz],
                                             start=(c == 0), stop=(c == CT - 1))
                        ot = tpool.tile([128, 128], f32, tag="ot")
                        nc.vector.tensor_copy(out=ot[:, :isz], in_=o_ps[:, :isz])
                        odst = bass.AP(tensor=outT_d, offset=128 * f * ipc + i0,
                                       ap=[[ipc, 128], [1, isz]])
                        nc.sync.dma_start(out=odst, in_=ot[:, :isz])
    nc.compile()
    return nc


def _get_prog(ipc=IPC, mask_ones=True):
    key = (ipc, mask_ones)
    if key not in _prog_cache:
        _prog_cache[key] = _build(ipc, mask_ones)
    return _prog_cache[key]


def _in_maps(s, z, mask, k_in, Wq, bq, Wk, Wv, Wg, ln_g, ln_b, Wz, Wo, ipc=IPC):
    import ml_dtypes
    del ln_b  # constant along j after softmax -> drops out exactly
    bf = ml_dtypes.bfloat16
    maps = []
    nsl = NCORES // B
    zbf = np.asarray(z, np.float32).astype(bf)
    Wbf = {n: np.asarray(w, np.float32).astype(bf)
           for n, w in (("Wq", Wq), ("Wk", Wk), ("Wv", Wv), ("Wg", Wg), ("Wo", Wo))}
    mask_is_ones = bool(np.all(np.asarray(mask) == 1.0))
    for c in range(NCORES):
        b, t = divmod(c, nsl)
        i0 = t * ipc
        m = {
            "zT": np.ascontiguousarray(
                zbf[b, i0:i0 + ipc].transpose(0, 2, 1)).reshape(ipc * CZ, N),
            "sT": np.ascontiguousarray(
                np.asarray(s, np.float32)[b, i0:i0 + ipc].T).astype(bf),
            "kinT": np.ascontiguousarray(
                np.asarray(k_in, np.float32)[b].T).astype(bf),
            "Wq": Wbf["Wq"], "Wk": Wbf["Wk"], "Wv": Wbf["Wv"],
            "Wg": Wbf["Wg"], "Wo": Wbf["Wo"],
            "Wz": np.ascontiguousarray(np.asarray(Wz, np.float32)),
            "ln_g": np.ascontiguousarray(
                np.asarray(ln_g, np.float32).reshape(CZ, 1)),
            "bqT": np.ascontiguousarray(
                np.asarray(bq, np.float32).reshape(CT_host, 128).T),
        }
        if not mask_is_ones:
            m["maskrow"] = np.ascontiguousarray(
                np.asarray(mask, np.float32)[b].reshape(1, N))
        maps.append(m)
    return maps


def kernel(**inputs):
    from concourse.bass_utils import run_bass_kernel_spmd
    mask_ones = bool(np.all(np.asarray(inputs["mask"]) == 1.0))
    nc = _get_prog(IPC, mask_ones)
    maps = _in_maps(**{k: np.asarray(v) for k, v in inputs.items()})
    trace = os.environ.get("KBENCH_TRACE", "") == "1"
    res = run_bass_kernel_spmd(nc, maps, core_ids=list(range(NCORES)), trace=trace)
    out = np.empty((B, N, CS), dtype=np.float32)
    nsl = NCORES // B
    for c in range(NCORES):
        b, t = divmod(c, nsl)
        out[b, t * IPC:(t + 1) * IPC, :] = res.results[c]["outT"].T
    if trace:
        print("HW exec time:", res.exec_time_ns, "ns")
    return out



# revision 45
# speedup vs baseline: 1.1376x; 1.1376x over previous
"""AttentionPairBias kernel for Trainium2, 8-core SPMD.

Problem (reference.py):
  q = (s @ Wq + bq); k = k_in @ Wk; v = k_in @ Wv       (B,N,H,D)
  zn = LayerNorm(z) * ln_g + ln_b                        (B,N,N,CZ)
  bias = transpose(zn @ Wz) -> (B,H,N,N)
  g = sigmoid(s @ Wg)
  p = softmax_j(q.k/sqrt(D) + bias + maskterm)
  out = (g * (p @ v)) @ Wo

Sharding: 8 cores = (batch b in {0,1}) x (4 slices of 192 query rows i).
Each core computes out[b, i0:i0+192, :] completely (full key range
on-core), so the host only slices inputs / concatenates outputs.
Host-side prep is layout-only: slicing, transposes (sT, kinT, per-row zT).

Device math restructuring:
  LayerNorm folded into the Wz projection:
    bias[pair,h] = rstd[pair] * (z[pair,:] @ W''[:,h])  (+ per-(b,h,i) const
    dropped -- softmax over j is invariant to it; this absorbs ln_b and the
    mean term)
  W'[c,h] = ln_g[c]*Wz[c,h];  W''[c,h] = W'[c,h] - mean_c W'[:,h]
  rstd from per-pair sum(z) (ones-column riding the same matmul) and
  sum(z^2) (ones-matmul over squared z).
"""

import os
import numpy as np

B, N, CS, CZ, H, D = 2, 768, 1024, 128, 16, 64
NCORES = 8
IPC = N // 4            # 192 query rows per core
CT_host = CS // 128
LN_EPS = 1e-5
INV_SQRT_D = 0.125
INF = 1000000.0

_prog_cache = {}


def _build(ipc=IPC, mask_ones=True, repeat=1):
    import contextlib
    import concourse.bass as bass
    import concourse.tile as tile
    from concourse import bacc, mybir

    f32 = mybir.dt.float32
    bf16 = mybir.dt.bfloat16
    AF = mybir.ActivationFunctionType
    OP = mybir.AluOpType

    CT = CS // 128          # 8 c_s tiles
    JT = N // 128           # 6 j tiles
    assert ipc % 4 == 0
    itiles = []
    o = 0
    while o < ipc:
        isz = min(128, ipc - o)
        itiles.append((o, isz, 0))
        o += isz

    nc = bacc.Bacc("TRN2", target_bir_lowering=False, debug=False,
                   enable_asserts=False, num_devices=NCORES)

    # ---- DRAM I/O ----
    # z/s/k_in/W* ship as bf16 (host cast): halves HBM traffic; compute
    # was bf16 in SBUF already.
    zT_d = nc.dram_tensor("zT", [ipc * CZ, N], bf16, kind="ExternalInput")
    sT_d = nc.dram_tensor("sT", [CS, ipc], bf16, kind="ExternalInput")
    kinT_d = nc.dram_tensor("kinT", [CS, N], bf16, kind="ExternalInput")
    W_d = {w: nc.dram_tensor(w, [CS, CS], bf16, kind="ExternalInput")
           for w in ("Wq", "Wk", "Wv", "Wg", "Wo")}
    Wz_d = nc.dram_tensor("Wz", [CZ, H], f32, kind="ExternalInput")
    lng_d = nc.dram_tensor("ln_g", [CZ, 1], f32, kind="ExternalInput")
    bqT_d = nc.dram_tensor("bqT", [128, CT], f32, kind="ExternalInput")
    if not mask_ones:
        mask_d = nc.dram_tensor("maskrow", [1, N], f32, kind="ExternalInput")
    outT_d = nc.dram_tensor("outT", [CS, ipc], f32, kind="ExternalOutput")

    with tile.TileContext(nc) as tc:
        # ---------------- persistent SBUF ----------------
        Wsb = {w: nc.alloc_sbuf_tensor(f"{w}_sb", [128, CT, CS], bf16)
               for w in ("Wq", "Wk", "Wv", "Wg", "Wo")}
        kinT_sb = nc.alloc_sbuf_tensor("kinT_sb", [128, CT, N], bf16)
        sT_sb = nc.alloc_sbuf_tensor("sT_sb", [128, CT, ipc], bf16)
        qT_sb = nc.alloc_sbuf_tensor("qT_sb", [128, CT, ipc], bf16)
        kT_sb = nc.alloc_sbuf_tensor("kT_sb", [128, CT, N], bf16)
        v_sb = nc.alloc_sbuf_tensor("v_sb", [128, JT, CS], bf16)
        gT_sb = nc.alloc_sbuf_tensor("gT_sb", [128, CT, ipc], bf16)
        goT_sb = nc.alloc_sbuf_tensor("goT_sb", [128, CT, 128], bf16)
        bqT_sb = nc.alloc_sbuf_tensor("bqT_sb", [128, CT], f32)
        Wz_sb = nc.alloc_sbuf_tensor("Wz_sb", [128, H], f32)
        lng_sb = nc.alloc_sbuf_tensor("lng_sb", [128, 1], f32)
        Wp_sb = nc.alloc_sbuf_tensor("Wp_sb", [128, H], f32)
        G_sb = nc.alloc_sbuf_tensor("G_sb", [128, H], f32)
        Grow_sb = nc.alloc_sbuf_tensor("Grow_sb", [1, H], f32)
        Waug = nc.alloc_sbuf_tensor("Waug", [128, 32], bf16)
        ones17 = nc.alloc_sbuf_tensor("ones17", [128, 32], bf16)
        id_sb = nc.alloc_sbuf_tensor("id_sb", [128, 128], bf16)
        eps_sb = nc.alloc_sbuf_tensor("eps_sb", [128, 1], f32)
        # per-i-tile working buffers: rows 0..15 = P, 16 = sum z, 17 = sum z^2
        PH_nat = nc.alloc_sbuf_tensor("PH_nat", [128, H + 2, N], bf16)
        stat_a = nc.alloc_sbuf_tensor("stat_a", [128, N], f32)
        stat_b = nc.alloc_sbuf_tensor("stat_b", [128, N], f32)
        alpha = nc.alloc_sbuf_tensor("alpha", [128, N], bf16)
        den_sb = nc.alloc_sbuf_tensor("den_sb", [128, 2 * H], f32)
        rden_sb = nc.alloc_sbuf_tensor("rden_sb", [128, 2 * H], f32)
        if not mask_ones:
            mrow_sb = nc.alloc_sbuf_tensor("mrow_sb", [1, N], f32)
            mbias_sb = nc.alloc_sbuf_tensor("mbias_sb", [1, N], f32)
            mb_full = nc.alloc_sbuf_tensor("mb_full", [128, N], bf16)

        ctx = contextlib.ExitStack()
        with ctx:
            ps = ctx.enter_context(tc.tile_pool(name="ps", bufs=8, space="PSUM"))
            zpool = ctx.enter_context(tc.tile_pool(name="zs", bufs=4))
            tpool = ctx.enter_context(tc.tile_pool(name="ts", bufs=2))

            # ---------------- constant / weight prep ----------------
            nc.sync.dma_start(out=bqT_sb[:, :], in_=bqT_d.ap())
            nc.sync.dma_start(out=Wz_sb[:, :], in_=Wz_d.ap())
            nc.sync.dma_start(out=lng_sb[:, :], in_=lng_d.ap())
            nc.vector.memset(eps_sb[:, :], LN_EPS)
            # ones17: one-hot ones at col 17 -> S matmul accumulates sum(z^2)
            # into row 17 of the P matmul's PSUM tile (rows 0..16 += 0)
            nc.vector.memset(ones17[:, :], 0.0)
            nc.vector.memset(ones17[:, 17:18], 1.0)
            nc.vector.memset(Waug[:, :], 0.0)
            from concourse.masks import make_identity
            make_identity(nc, id_sb[:, :])
            # W' = ln_g*Wz ; G = colsum(W') ; W'' = W' - G/128
            nc.vector.tensor_scalar_mul(out=Wp_sb[:, :], in0=Wz_sb[:, :],
                                        scalar1=lng_sb[:, :])
            nc.vector.tensor_copy(out=Waug[:, 0:H], in_=Wp_sb[:, :])
            nc.vector.memset(Waug[:, H:H + 1], 1.0)
            G_ps = ps.tile([1, H], f32, tag="bank")
            nc.tensor.matmul(out=G_ps[:, :], lhsT=Waug[:, H:H + 1],
                             rhs=Waug[:, 0:H], start=True, stop=True)
            nc.vector.tensor_copy(out=Grow_sb[:, :], in_=G_ps[:, :])
            Grow_dram = nc.dram_tensor("Grow_dram", [H], f32, kind="Internal")
            nc.sync.dma_start(out=Grow_dram.ap(), in_=Grow_sb[:, :])
            nc.sync.dma_start(
                out=G_sb[:, :],
                in_=bass.AP(tensor=Grow_dram, offset=0, ap=[[0, 128], [1, H]]))
            nc.vector.scalar_tensor_tensor(out=Wp_sb[:, :], in0=G_sb[:, :],
                                           scalar=-1.0 / 128.0, in1=Wp_sb[:, :],
                                           op0=OP.mult, op1=OP.add)
            nc.vector.tensor_copy(out=Waug[:, 0:H], in_=Wp_sb[:, :])
            if not mask_ones:
                nc.sync.dma_start(out=mrow_sb[:, :], in_=mask_d.ap())
                nc.vector.tensor_scalar(out=mbias_sb[:, :], in0=mrow_sb[:, :],
                                        scalar1=1.0, scalar2=INF,
                                        op0=OP.subtract, op1=OP.mult)
                mb_dram = nc.dram_tensor("mb_dram", [N], f32, kind="Internal")
                nc.sync.dma_start(out=mb_dram.ap(), in_=mbias_sb[:, :])
                nc.gpsimd.dma_start(
                    out=mb_full[:, :],
                    in_=bass.AP(tensor=mb_dram, offset=0, ap=[[0, 128], [1, N]]))

            # weight / activation loads (bf16 in DRAM already) on the scalar
            # HWDGE ring, which is idle at kernel start -- keeps the 12.4MB
            # weight burst from head-of-line blocking the gpsimd z stream
            for w in ("Wq", "Wk", "Wv", "Wg", "Wo"):
                nc.scalar.dma_start(
                    out=Wsb[w][:, :, :],
                    in_=W_d[w].ap().rearrange("(t p) f -> p t f", p=128))
            nc.scalar.dma_start(
                out=kinT_sb[:, :, :],
                in_=kinT_d.ap().rearrange("(t p) j -> p t j", p=128))
            nc.scalar.dma_start(
                out=sT_sb[:, :, :],
                in_=sT_d.ap().rearrange("(t p) i -> p t i", p=128))

            for _rep in range(repeat):
                # ---------------- projections ----------------
                for f in range(CT):
                    g_ps = ps.tile([128, ipc], f32, tag="bank")
                    for c in range(CT):
                        nc.tensor.matmul(out=g_ps[:, :],
                                         lhsT=Wsb["Wg"][:, c, 128 * f:128 * (f + 1)],
                                         rhs=sT_sb[:, c, :],
                                         start=(c == 0), stop=(c == CT - 1))
                    nc.scalar.activation(out=gT_sb[:, f, :], in_=g_ps[:, :], func=AF.Sigmoid)
                for f in range(CT):
                    q_ps = ps.tile([128, ipc], f32, tag="bank")
                    for c in range(CT):
                        nc.tensor.matmul(out=q_ps[:, :],
                                         lhsT=Wsb["Wq"][:, c, 128 * f:128 * (f + 1)],
                                         rhs=sT_sb[:, c, :],
                                         start=(c == 0), stop=(c == CT - 1))
                    nc.vector.tensor_scalar_add(out=qT_sb[:, f, :], in0=q_ps[:, :],
                                                scalar1=bqT_sb[:, f:f + 1])
                for f in range(CT):
                    for hf in range(2):
                        k_ps = ps.tile([128, 384], f32, tag="bank")
                        for c in range(CT):
                            nc.tensor.matmul(out=k_ps[:, :],
                                             lhsT=Wsb["Wk"][:, c, 128 * f:128 * (f + 1)],
                                             rhs=kinT_sb[:, c, 384 * hf:384 * (hf + 1)],
                                             start=(c == 0), stop=(c == CT - 1))
                        nc.vector.tensor_copy(out=kT_sb[:, f, 384 * hf:384 * (hf + 1)],
                                              in_=k_ps[:, :])
                for jt in range(JT):
                    for hf in range(2):
                        v_ps = ps.tile([128, 512], f32, tag="bank")
                        for c in range(CT):
                            nc.tensor.matmul(out=v_ps[:, :],
                                             lhsT=kinT_sb[:, c, 128 * jt:128 * (jt + 1)],
                                             rhs=Wsb["Wv"][:, c, 512 * hf:512 * (hf + 1)],
                                             start=(c == 0), stop=(c == CT - 1))
                        nc.vector.tensor_copy(out=v_sb[:, jt, 512 * hf:512 * (hf + 1)],
                                              in_=v_ps[:, :])

                # ---------------- main loop over i-tiles ----------------
                for (i0, isz, poff) in itiles:
                    # ---- z stream: 4 i-rows per DMA (host-pretransposed bf16 zT)
                    # on the gpsimd SWDGE ring, decoupled from the small
                    # HWDGE transpose DMAs ----
                    for r4 in range(i0, i0 + isz, 4):
                        zTt = zpool.tile([128, 4, N], bf16, tag="zTt")
                        zsrc = bass.AP(
                            tensor=zT_d,
                            offset=r4 * CZ * N,
                            ap=[[N, 128], [CZ * N, 4], [1, N]],
                        )
                        eng_z = nc.gpsimd if (r4 // 4) % 2 == 0 else nc.sync
                        eng_z.dma_start(out=zTt[:, :, :], in_=zsrc)
                        Ppk = [ps.tile([128, 384], f32, tag="bank", name=f"Ppk{_h}")
                               for _h in range(2)]
                        for rl in range(4):
                            zsqT = zpool.tile([128, N], bf16, tag="zsqT")
                            if rl % 2 == 0:
                                nc.scalar.activation(out=zsqT[:, :], in_=zTt[:, rl, :],
                                                     func=AF.Square)
                            else:
                                nc.vector.scalar_tensor_tensor(
                                    out=zsqT[:, :], in0=zTt[:, rl, :], scalar=1.0,
                                    in1=zTt[:, rl, :], op0=OP.mult, op1=OP.mult)
                            for hf in range(2):
                                nc.tensor.matmul(out=Ppk[hf][32 * rl:32 * rl + 18, :],
                                                 lhsT=Waug[:, 0:18],
                                                 rhs=zTt[:, rl, 384 * hf:384 * (hf + 1)],
                                                 start=True, stop=False,
                                                 tile_position=(0, 32 * rl))
                                nc.tensor.matmul(out=Ppk[hf][32 * rl:32 * rl + 18, :],
                                                 lhsT=ones17[:, 0:18],
                                                 rhs=zsqT[:, 384 * hf:384 * (hf + 1)],
                                                 start=False, stop=True,
                                                 tile_position=(0, 32 * rl))
                        PSsb = tpool.tile([128, 2, 384], bf16, tag="PSsb")
                        for hf in range(2):
                            nc.vector.tensor_copy(out=PSsb[:, hf, :],
                                                  in_=Ppk[hf][:, :])
                        for rl in range(4):
                            li = r4 + rl - i0 + poff
                            eng_p = nc.sync if rl % 2 == 0 else nc.scalar
                            eng_p.dma_start(out=PH_nat[li:li + 1, :, :],
                                            in_=PSsb[32 * rl:32 * rl + H + 2, :, :])

                    # ---- stats -> alpha ; bias = alpha * P (in place) ----
                    sl = slice(poff, poff + isz)
                    nc.vector.scalar_tensor_tensor(
                        out=stat_a[sl, :], in0=PH_nat[sl, H, :],
                        scalar=1.0 / (128.0 * 128.0), in1=PH_nat[sl, H, :],
                        op0=OP.mult, op1=OP.mult)
                    nc.vector.scalar_tensor_tensor(
                        out=stat_b[sl, :], in0=PH_nat[sl, H + 1, :], scalar=1.0 / 128.0,
                        in1=stat_a[sl, :], op0=OP.mult, op1=OP.subtract)
                    # rstd = exp(-0.5*ln(var+eps)) -- keeps ACT on one table set
                    nc.scalar.activation(out=stat_b[sl, :], in_=stat_b[sl, :],
                                         func=AF.Ln, bias=eps_sb[sl, :], scale=1.0)
                    nc.scalar.activation(out=alpha[sl, :], in_=stat_b[sl, :],
                                         func=AF.Exp, scale=-0.5)

                    # ---- attention ----
                    for h in range(H):
                        hp, off = h // 2, 64 * (h % 2)
                        nc.vector.scalar_tensor_tensor(
                            out=PH_nat[sl, h, :], in0=PH_nat[sl, h, :], scalar=1.0,
                            in1=alpha[sl, :], op0=OP.mult, op1=OP.mult)
                        if not mask_ones:
                            nc.vector.tensor_add(out=PH_nat[sl, h, :],
                                                 in0=PH_nat[sl, h, :],
                                                 in1=mb_full[sl, :])
                        qk0 = ps.tile([128, 384], f32, tag="bank")
                        qk1 = ps.tile([128, 384], f32, tag="bank")
                        p_sb = tpool.tile([128, N], bf16, tag="p_sb")
                        for hf, qk in ((0, qk0), (1, qk1)):
                            nc.tensor.matmul(out=qk[sl, :],
                                             lhsT=qT_sb[off:off + 64, hp, i0:i0 + isz],
                                             rhs=kT_sb[off:off + 64, hp, 384 * hf:384 * (hf + 1)],
                                             start=True, stop=True,
                                             tile_position=(0, poff) if poff else None)
                            nc.vector.scalar_tensor_tensor(
                                out=qk[sl, :], in0=qk[sl, :], scalar=INV_SQRT_D,
                                in1=PH_nat[sl, h, 384 * hf:384 * (hf + 1)],
                                op0=OP.mult, op1=OP.add)
                            nc.scalar.activation(out=p_sb[sl, 384 * hf:384 * (hf + 1)],
                                                 in_=qk[sl, :], func=AF.Exp,
                                                 accum_out=den_sb[sl, 2 * h + hf:2 * h + hf + 1])
                        nc.vector.tensor_add(out=den_sb[sl, 2 * h:2 * h + 1],
                                             in0=den_sb[sl, 2 * h:2 * h + 1],
                                             in1=den_sb[sl, 2 * h + 1:2 * h + 2])
                        nc.vector.reciprocal(out=rden_sb[sl, h:h + 1],
                                             in_=den_sb[sl, 2 * h:2 * h + 1])
                        nc.vector.tensor_scalar_mul(out=p_sb[sl, :], in0=p_sb[sl, :],
                                                    scalar1=rden_sb[sl, h:h + 1])
                        pT_ps = ps.tile([128, JT, 128], bf16, tag="bank")
                        for jt in range(JT):
                            nc.tensor.transpose(out=pT_ps[:, jt, :isz],
                                                in_=p_sb[sl, 128 * jt:128 * (jt + 1)],
                                                identity=id_sb[sl, sl])
                        pT = tpool.tile([128, JT, 128], bf16, tag="pT")
                        nc.vector.tensor_copy(out=pT[:, :, :isz], in_=pT_ps[:, :, :isz])
                        if h % 2 == 0:
                            oT_ps = ps.tile([128, 128], f32, tag="bank")
                        for jt in range(JT):
                            nc.tensor.matmul(out=oT_ps[off:off + 64, :isz],
                                             lhsT=v_sb[:, jt, 64 * h:64 * (h + 1)],
                                             rhs=pT[:, jt, :isz],
                                             start=(jt == 0), stop=(jt == JT - 1))
                        if h % 2 == 1:
                            nc.vector.tensor_mul(out=goT_sb[:, hp, :isz],
                                                 in0=oT_ps[:, :isz],
                                                 in1=gT_sb[:, hp, i0:i0 + isz])

                    # ---- output projection ----
                    for f in range(CT):
                        o_ps = ps.tile([128, 128], f32, tag="bank")
                        for c in range(CT):
                            nc.tensor.matmul(out=o_ps[:, :isz],
                                             lhsT=Wsb["Wo"][:, c, 128 * f:128 * (f + 1)],
                                             rhs=goT_sb[:, c, :isz],
                                             start=(c == 0), stop=(c == CT - 1))
                        ot = tpool.tile([128, 128], f32, tag="ot")
                        nc.vector.tensor_copy(out=ot[:, :isz], in_=o_ps[:, :isz])
                        odst = bass.AP(tensor=outT_d, offset=128 * f * ipc + i0,
                                       ap=[[ipc, 128], [1, isz]])
                        nc.sync.dma_start(out=odst, in_=ot[:, :isz])
    nc.compile()
    return nc


def _get_prog(ipc=IPC, mask_ones=True):
    key = (ipc, mask_ones)
    if key not in _prog_cache:
        _prog_cache[key] = _build(ipc, mask_ones)
    return _prog_cache[key]


def _in_maps(s, z, mask, k_in, Wq, bq, Wk, Wv, Wg, ln_g, ln_b, Wz, Wo, ipc=IPC):
    import ml_dtypes
    del ln_b  # constant along j after softmax -> drops out exactly
    bf = ml_dtypes.bfloat16
    maps = []
    nsl = NCORES // B
    zbf = np.asarray(z, np.float32).astype(bf)
    Wbf = {n: np.asarray(w, np.float32).astype(bf)
           for n, w in (("Wq", Wq), ("Wk", Wk), ("Wv", Wv), ("Wg", Wg), ("Wo", Wo))}
    mask_is_ones = bool(np.all(np.asarray(mask) == 1.0))
    for c in range(NCORES):
        b, t = divmod(c, nsl)
        i0 = t * ipc
        m = {
            "zT": np.ascontiguousarray(
                zbf[b, i0:i0 + ipc].transpose(0, 2, 1)).reshape(ipc * CZ, N),
            "sT": np.ascontiguousarray(
                np.asarray(s, np.float32)[b, i0:i0 + ipc].T).astype(bf),
            "kinT": np.ascontiguousarray(
                np.asarray(k_in, np.float32)[b].T).astype(bf),
            "Wq": Wbf["Wq"], "Wk": Wbf["Wk"], "Wv": Wbf["Wv"],
            "Wg": Wbf["Wg"], "Wo": Wbf["Wo"],
            "Wz": np.ascontiguousarray(np.asarray(Wz, np.float32)),
            "ln_g": np.ascontiguousarray(
                np.asarray(ln_g, np.float32).reshape(CZ, 1)),
            "bqT": np.ascontiguousarray(
                np.asarray(bq, np.float32).reshape(CT_host, 128).T),
        }
        if not mask_is_ones:
            m["maskrow"] = np.ascontiguousarray(
                np.asarray(mask, np.float32)[b].reshape(1, N))
        maps.append(m)
    return maps


def kernel(**inputs):
    from concourse.bass_utils import run_bass_kernel_spmd
    mask_ones = bool(np.all(np.asarray(inputs["mask"]) == 1.0))
    nc = _get_prog(IPC, mask_ones)
    maps = _in_maps(**{k: np.asarray(v) for k, v in inputs.items()})
    trace = os.environ.get("KBENCH_TRACE", "") == "1"
    res = run_bass_kernel_spmd(nc, maps, core_ids=list(range(NCORES)), trace=trace)
    out = np.empty((B, N, CS), dtype=np.float32)
    nsl = NCORES // B
    for c in range(NCORES):
        b, t = divmod(c, nsl)
        out[b, t * IPC:(t + 1) * IPC, :] = res.results[c]["outT"].T
    if trace:
        print("HW exec time:", res.exec_time_ns, "ns")
    return out



# revision 46
# speedup vs baseline: 1.1558x; 1.0160x over previous
"""AttentionPairBias kernel for Trainium2, 8-core SPMD.

Problem (reference.py):
  q = (s @ Wq + bq); k = k_in @ Wk; v = k_in @ Wv       (B,N,H,D)
  zn = LayerNorm(z) * ln_g + ln_b                        (B,N,N,CZ)
  bias = transpose(zn @ Wz) -> (B,H,N,N)
  g = sigmoid(s @ Wg)
  p = softmax_j(q.k/sqrt(D) + bias + maskterm)
  out = (g * (p @ v)) @ Wo

Sharding: 8 cores = (batch b in {0,1}) x (4 slices of 192 query rows i).
Each core computes out[b, i0:i0+192, :] completely (full key range
on-core), so the host only slices inputs / concatenates outputs.
Host-side prep is layout-only: slicing, transposes (sT, kinT, per-row zT).

Device math restructuring:
  LayerNorm folded into the Wz projection:
    bias[pair,h] = rstd[pair] * (z[pair,:] @ W''[:,h])  (+ per-(b,h,i) const
    dropped -- softmax over j is invariant to it; this absorbs ln_b and the
    mean term)
  W'[c,h] = ln_g[c]*Wz[c,h];  W''[c,h] = W'[c,h] - mean_c W'[:,h]
  rstd from per-pair sum(z) (ones-column riding the same matmul) and
  sum(z^2) (ones-matmul over squared z).
"""

import os
import numpy as np

B, N, CS, CZ, H, D = 2, 768, 1024, 128, 16, 64
NCORES = 8
IPC = N // 4            # 192 query rows per core
CT_host = CS // 128
LN_EPS = 1e-5
INV_SQRT_D = 0.125
INF = 1000000.0

_prog_cache = {}


def _build(ipc=IPC, mask_ones=True, repeat=1):
    import contextlib
    import concourse.bass as bass
    import concourse.tile as tile
    from concourse import bacc, mybir

    f32 = mybir.dt.float32
    bf16 = mybir.dt.bfloat16
    AF = mybir.ActivationFunctionType
    OP = mybir.AluOpType

    CT = CS // 128          # 8 c_s tiles
    JT = N // 128           # 6 j tiles
    assert ipc % 4 == 0
    itiles = []
    o = 0
    while o < ipc:
        isz = min(128, ipc - o)
        itiles.append((o, isz, 0))
        o += isz

    nc = bacc.Bacc("TRN2", target_bir_lowering=False, debug=False,
                   enable_asserts=False, num_devices=NCORES)

    # ---- DRAM I/O ----
    # z/s/k_in/W* ship as bf16 (host cast): halves HBM traffic; compute
    # was bf16 in SBUF already.
    zT_d = nc.dram_tensor("zT", [ipc * CZ, N], bf16, kind="ExternalInput")
    sT_d = nc.dram_tensor("sT", [CS, ipc], bf16, kind="ExternalInput")
    kinT_d = nc.dram_tensor("kinT", [CS, N], bf16, kind="ExternalInput")
    W_d = {w: nc.dram_tensor(w, [CS, CS], bf16, kind="ExternalInput")
           for w in ("Wq", "Wk", "Wv", "Wg", "Wo")}
    Wz_d = nc.dram_tensor("Wz", [CZ, H], f32, kind="ExternalInput")
    lng_d = nc.dram_tensor("ln_g", [CZ, 1], f32, kind="ExternalInput")
    bqT_d = nc.dram_tensor("bqT", [128, CT], f32, kind="ExternalInput")
    if not mask_ones:
        mask_d = nc.dram_tensor("maskrow", [1, N], f32, kind="ExternalInput")
    outT_d = nc.dram_tensor("outT", [CS, ipc], f32, kind="ExternalOutput")

    with tile.TileContext(nc) as tc:
        # ---------------- persistent SBUF ----------------
        Wsb = {w: nc.alloc_sbuf_tensor(f"{w}_sb", [128, CT, CS], bf16)
               for w in ("Wq", "Wk", "Wv", "Wg", "Wo")}
        kinT_sb = nc.alloc_sbuf_tensor("kinT_sb", [128, CT, N], bf16)
        sT_sb = nc.alloc_sbuf_tensor("sT_sb", [128, CT, ipc], bf16)
        qT_sb = nc.alloc_sbuf_tensor("qT_sb", [128, CT, ipc], bf16)
        kT_sb = nc.alloc_sbuf_tensor("kT_sb", [128, CT, N], bf16)
        v_sb = nc.alloc_sbuf_tensor("v_sb", [128, JT, CS], bf16)
        gT_sb = nc.alloc_sbuf_tensor("gT_sb", [128, CT, ipc], bf16)
        goT_sb = nc.alloc_sbuf_tensor("goT_sb", [128, CT, 128], bf16)
        bqT_sb = nc.alloc_sbuf_tensor("bqT_sb", [128, CT], f32)
        Wz_sb = nc.alloc_sbuf_tensor("Wz_sb", [128, H], f32)
        lng_sb = nc.alloc_sbuf_tensor("lng_sb", [128, 1], f32)
        Wp_sb = nc.alloc_sbuf_tensor("Wp_sb", [128, H], f32)
        G_sb = nc.alloc_sbuf_tensor("G_sb", [128, H], f32)
        Grow_sb = nc.alloc_sbuf_tensor("Grow_sb", [1, H], f32)
        Waug = nc.alloc_sbuf_tensor("Waug", [128, 32], bf16)
        ones17 = nc.alloc_sbuf_tensor("ones17", [128, 32], bf16)
        id_sb = nc.alloc_sbuf_tensor("id_sb", [128, 128], bf16)
        eps_sb = nc.alloc_sbuf_tensor("eps_sb", [128, 1], f32)
        # per-i-tile working buffers: rows 0..15 = P, 16 = sum z, 17 = sum z^2
        PH_nat = nc.alloc_sbuf_tensor("PH_nat", [128, H + 2, N], bf16)
        stat_a = nc.alloc_sbuf_tensor("stat_a", [128, N], f32)
        stat_b = nc.alloc_sbuf_tensor("stat_b", [128, N], f32)
        alpha = nc.alloc_sbuf_tensor("alpha", [128, N], bf16)
        den_sb = nc.alloc_sbuf_tensor("den_sb", [128, 2 * H], f32)
        rden_sb = nc.alloc_sbuf_tensor("rden_sb", [128, 2 * H], f32)
        if not mask_ones:
            mrow_sb = nc.alloc_sbuf_tensor("mrow_sb", [1, N], f32)
            mbias_sb = nc.alloc_sbuf_tensor("mbias_sb", [1, N], f32)
            mb_full = nc.alloc_sbuf_tensor("mb_full", [128, N], bf16)

        ctx = contextlib.ExitStack()
        with ctx:
            ps = ctx.enter_context(tc.tile_pool(name="ps", bufs=8, space="PSUM"))
            zpool = ctx.enter_context(tc.tile_pool(name="zs", bufs=4))
            tpool = ctx.enter_context(tc.tile_pool(name="ts", bufs=2))

            # ---------------- constant / weight prep ----------------
            nc.sync.dma_start(out=bqT_sb[:, :], in_=bqT_d.ap())
            nc.sync.dma_start(out=Wz_sb[:, :], in_=Wz_d.ap())
            nc.sync.dma_start(out=lng_sb[:, :], in_=lng_d.ap())
            nc.vector.memset(eps_sb[:, :], LN_EPS)
            # ones17: one-hot ones at col 17 -> S matmul accumulates sum(z^2)
            # into row 17 of the P matmul's PSUM tile (rows 0..16 += 0)
            nc.vector.memset(ones17[:, :], 0.0)
            nc.vector.memset(ones17[:, 17:18], 1.0)
            nc.vector.memset(Waug[:, :], 0.0)
            from concourse.masks import make_identity
            make_identity(nc, id_sb[:, :])
            # W' = ln_g*Wz ; G = colsum(W') ; W'' = W' - G/128
            nc.vector.tensor_scalar_mul(out=Wp_sb[:, :], in0=Wz_sb[:, :],
                                        scalar1=lng_sb[:, :])
            nc.vector.tensor_copy(out=Waug[:, 0:H], in_=Wp_sb[:, :])
            nc.vector.memset(Waug[:, H:H + 1], 1.0)
            G_ps = ps.tile([1, H], f32, tag="bank")
            nc.tensor.matmul(out=G_ps[:, :], lhsT=Waug[:, H:H + 1],
                             rhs=Waug[:, 0:H], start=True, stop=True)
            nc.vector.tensor_copy(out=Grow_sb[:, :], in_=G_ps[:, :])
            Grow_dram = nc.dram_tensor("Grow_dram", [H], f32, kind="Internal")
            nc.sync.dma_start(out=Grow_dram.ap(), in_=Grow_sb[:, :])
            nc.sync.dma_start(
                out=G_sb[:, :],
                in_=bass.AP(tensor=Grow_dram, offset=0, ap=[[0, 128], [1, H]]))
            nc.vector.scalar_tensor_tensor(out=Wp_sb[:, :], in0=G_sb[:, :],
                                           scalar=-1.0 / 128.0, in1=Wp_sb[:, :],
                                           op0=OP.mult, op1=OP.add)
            nc.vector.tensor_copy(out=Waug[:, 0:H], in_=Wp_sb[:, :])
            if not mask_ones:
                nc.sync.dma_start(out=mrow_sb[:, :], in_=mask_d.ap())
                nc.vector.tensor_scalar(out=mbias_sb[:, :], in0=mrow_sb[:, :],
                                        scalar1=1.0, scalar2=INF,
                                        op0=OP.subtract, op1=OP.mult)
                mb_dram = nc.dram_tensor("mb_dram", [N], f32, kind="Internal")
                nc.sync.dma_start(out=mb_dram.ap(), in_=mbias_sb[:, :])
                nc.gpsimd.dma_start(
                    out=mb_full[:, :],
                    in_=bass.AP(tensor=mb_dram, offset=0, ap=[[0, 128], [1, N]]))

            # weight / activation loads (bf16 in DRAM already)
            for w in ("Wq", "Wk", "Wv", "Wg", "Wo"):
                nc.gpsimd.dma_start(
                    out=Wsb[w][:, :, :],
                    in_=W_d[w].ap().rearrange("(t p) f -> p t f", p=128))
            nc.gpsimd.dma_start(
                out=kinT_sb[:, :, :],
                in_=kinT_d.ap().rearrange("(t p) j -> p t j", p=128))
            nc.gpsimd.dma_start(
                out=sT_sb[:, :, :],
                in_=sT_d.ap().rearrange("(t p) i -> p t i", p=128))

            for _rep in range(repeat):
                # ---------------- projections ----------------
                for f in range(CT):
                    g_ps = ps.tile([128, ipc], f32, tag="bank")
                    for c in range(CT):
                        nc.tensor.matmul(out=g_ps[:, :],
                                         lhsT=Wsb["Wg"][:, c, 128 * f:128 * (f + 1)],
                                         rhs=sT_sb[:, c, :],
                                         start=(c == 0), stop=(c == CT - 1))
                    nc.scalar.activation(out=gT_sb[:, f, :], in_=g_ps[:, :], func=AF.Sigmoid)
                for f in range(CT):
                    q_ps = ps.tile([128, ipc], f32, tag="bank")
                    for c in range(CT):
                        nc.tensor.matmul(out=q_ps[:, :],
                                         lhsT=Wsb["Wq"][:, c, 128 * f:128 * (f + 1)],
                                         rhs=sT_sb[:, c, :],
                                         start=(c == 0), stop=(c == CT - 1))
                    nc.vector.tensor_scalar_add(out=qT_sb[:, f, :], in0=q_ps[:, :],
                                                scalar1=bqT_sb[:, f:f + 1])
                for f in range(CT):
                    for hf in range(2):
                        k_ps = ps.tile([128, 384], f32, tag="bank")
                        for c in range(CT):
                            nc.tensor.matmul(out=k_ps[:, :],
                                             lhsT=Wsb["Wk"][:, c, 128 * f:128 * (f + 1)],
                                             rhs=kinT_sb[:, c, 384 * hf:384 * (hf + 1)],
                                             start=(c == 0), stop=(c == CT - 1))
                        nc.vector.tensor_copy(out=kT_sb[:, f, 384 * hf:384 * (hf + 1)],
                                              in_=k_ps[:, :])
                for jt in range(JT):
                    for hf in range(2):
                        v_ps = ps.tile([128, 512], f32, tag="bank")
                        for c in range(CT):
                            nc.tensor.matmul(out=v_ps[:, :],
                                             lhsT=kinT_sb[:, c, 128 * jt:128 * (jt + 1)],
                                             rhs=Wsb["Wv"][:, c, 512 * hf:512 * (hf + 1)],
                                             start=(c == 0), stop=(c == CT - 1))
                        nc.vector.tensor_copy(out=v_sb[:, jt, 512 * hf:512 * (hf + 1)],
                                              in_=v_ps[:, :])

                # ---------------- main loop over i-tiles ----------------
                for (i0, isz, poff) in itiles:
                    # ---- z stream: 4 i-rows per DMA (host-pretransposed bf16 zT)
                    # on the gpsimd SWDGE ring, decoupled from the small
                    # HWDGE transpose DMAs ----
                    for r4 in range(i0, i0 + isz, 4):
                        zTt = zpool.tile([128, 4, N], bf16, tag="zTt")
                        zsrc = bass.AP(
                            tensor=zT_d,
                            offset=r4 * CZ * N,
                            ap=[[N, 128], [CZ * N, 4], [1, N]],
                        )
                        eng_z = nc.gpsimd if (r4 // 4) % 2 == 0 else nc.sync
                        eng_z.dma_start(out=zTt[:, :, :], in_=zsrc)
                        Ppk = [ps.tile([128, 384], f32, tag="bank", name=f"Ppk{_h}")
                               for _h in range(2)]
                        for rl in range(4):
                            zsqT = zpool.tile([128, N], bf16, tag="zsqT")
                            if rl % 2 == 0:
                                nc.scalar.activation(out=zsqT[:, :], in_=zTt[:, rl, :],
                                                     func=AF.Square)
                            else:
                                nc.vector.scalar_tensor_tensor(
                                    out=zsqT[:, :], in0=zTt[:, rl, :], scalar=1.0,
                                    in1=zTt[:, rl, :], op0=OP.mult, op1=OP.mult)
                            for hf in range(2):
                                nc.tensor.matmul(out=Ppk[hf][32 * rl:32 * rl + 18, :],
                                                 lhsT=Waug[:, 0:18],
                                                 rhs=zTt[:, rl, 384 * hf:384 * (hf + 1)],
                                                 start=True, stop=False,
                                                 tile_position=(0, 32 * rl))
                                nc.tensor.matmul(out=Ppk[hf][32 * rl:32 * rl + 18, :],
                                                 lhsT=ones17[:, 0:18],
                                                 rhs=zsqT[:, 384 * hf:384 * (hf + 1)],
                                                 start=False, stop=True,
                                                 tile_position=(0, 32 * rl))
                        PSsb = tpool.tile([128, 2, 384], bf16, tag="PSsb")
                        for hf in range(2):
                            nc.vector.tensor_copy(out=PSsb[:, hf, :],
                                                  in_=Ppk[hf][:, :])
                        for rl in range(4):
                            li = r4 + rl - i0 + poff
                            eng_p = nc.sync if rl % 2 == 0 else nc.scalar
                            eng_p.dma_start(out=PH_nat[li:li + 1, :, :],
                                            in_=PSsb[32 * rl:32 * rl + H + 2, :, :])

                    # ---- stats -> alpha ; bias = alpha * P (in place) ----
                    sl = slice(poff, poff + isz)
                    nc.vector.scalar_tensor_tensor(
                        out=stat_a[sl, :], in0=PH_nat[sl, H, :],
                        scalar=1.0 / (128.0 * 128.0), in1=PH_nat[sl, H, :],
                        op0=OP.mult, op1=OP.mult)
                    nc.vector.scalar_tensor_tensor(
                        out=stat_b[sl, :], in0=PH_nat[sl, H + 1, :], scalar=1.0 / 128.0,
                        in1=stat_a[sl, :], op0=OP.mult, op1=OP.subtract)
                    # rstd = exp(-0.5*ln(var+eps)) -- keeps ACT on one table set
                    nc.scalar.activation(out=stat_b[sl, :], in_=stat_b[sl, :],
                                         func=AF.Ln, bias=eps_sb[sl, :], scale=1.0)
                    nc.scalar.activation(out=alpha[sl, :], in_=stat_b[sl, :],
                                         func=AF.Exp, scale=-0.5)

                    # ---- attention ----
                    for h in range(H):
                        hp, off = h // 2, 64 * (h % 2)
                        nc.vector.scalar_tensor_tensor(
                            out=PH_nat[sl, h, :], in0=PH_nat[sl, h, :], scalar=1.0,
                            in1=alpha[sl, :], op0=OP.mult, op1=OP.mult)
                        if not mask_ones:
                            nc.vector.tensor_add(out=PH_nat[sl, h, :],
                                                 in0=PH_nat[sl, h, :],
                                                 in1=mb_full[sl, :])
                        qk0 = ps.tile([128, 384], f32, tag="bank")
                        qk1 = ps.tile([128, 384], f32, tag="bank")
                        p_sb = tpool.tile([128, N], bf16, tag="p_sb")
                        for hf, qk in ((0, qk0), (1, qk1)):
                            nc.tensor.matmul(out=qk[sl, :],
                                             lhsT=qT_sb[off:off + 64, hp, i0:i0 + isz],
                                             rhs=kT_sb[off:off + 64, hp, 384 * hf:384 * (hf + 1)],
                                             start=True, stop=True,
                                             tile_position=(0, poff) if poff else None)
                            nc.vector.scalar_tensor_tensor(
                                out=qk[sl, :], in0=qk[sl, :], scalar=INV_SQRT_D,
                                in1=PH_nat[sl, h, 384 * hf:384 * (hf + 1)],
                                op0=OP.mult, op1=OP.add)
                            nc.scalar.activation(out=p_sb[sl, 384 * hf:384 * (hf + 1)],
                                                 in_=qk[sl, :], func=AF.Exp,
                                                 accum_out=den_sb[sl, 2 * h + hf:2 * h + hf + 1])
                        nc.vector.tensor_add(out=den_sb[sl, 2 * h:2 * h + 1],
                                             in0=den_sb[sl, 2 * h:2 * h + 1],
                                             in1=den_sb[sl, 2 * h + 1:2 * h + 2])
                        nc.vector.reciprocal(out=rden_sb[sl, h:h + 1],
                                             in_=den_sb[sl, 2 * h:2 * h + 1])
                        nc.vector.tensor_scalar_mul(out=p_sb[sl, :], in0=p_sb[sl, :],
                                                    scalar1=rden_sb[sl, h:h + 1])
                        pT_ps = ps.tile([128, JT, 128], bf16, tag="bank")
                        for jt in range(JT):
                            nc.tensor.transpose(out=pT_ps[:, jt, :isz],
                                                in_=p_sb[sl, 128 * jt:128 * (jt + 1)],
                                                identity=id_sb[sl, sl])
                        pT = tpool.tile([128, JT, 128], bf16, tag="pT")
                        nc.vector.tensor_copy(out=pT[:, :, :isz], in_=pT_ps[:, :, :isz])
                        if h % 2 == 0:
                            oT_ps = ps.tile([128, 128], f32, tag="bank")
                        for jt in range(JT):
                            nc.tensor.matmul(out=oT_ps[off:off + 64, :isz],
                                             lhsT=v_sb[:, jt, 64 * h:64 * (h + 1)],
                                             rhs=pT[:, jt, :isz],
                                             start=(jt == 0), stop=(jt == JT - 1))
                        if h % 2 == 1:
                            nc.vector.tensor_mul(out=goT_sb[:, hp, :isz],
                                                 in0=oT_ps[:, :isz],
                                                 in1=gT_sb[:, hp, i0:i0 + isz])

                    # ---- output projection ----
                    for f in range(CT):
                        o_ps = ps.tile([128, 128], f32, tag="bank")
                        for c in range(CT):
                            nc.tensor.matmul(out=o_ps[:, :isz],
                                             lhsT=Wsb["Wo"][:, c, 128 * f:128 * (f + 1)],
                                             rhs=goT_sb[:, c, :isz],
                                             start=(c == 0), stop=(c == CT - 1))
                        ot = tpool.tile([128, 128], f32, tag="ot")
                        nc.vector.tensor_copy(out=ot[:, :isz], in_=o_ps[:, :isz])
                        odst = bass.AP(tensor=outT_d, offset=128 * f * ipc + i0,
                                       ap=[[ipc, 128], [1, isz]])
                        nc.sync.dma_start(out=odst, in_=ot[:, :isz])
    nc.compile()
    return nc


def _get_prog(ipc=IPC, mask_ones=True):
    key = (ipc, mask_ones)
    if key not in _prog_cache:
        _prog_cache[key] = _build(ipc, mask_ones)
    return _prog_cache[key]


def _in_maps(s, z, mask, k_in, Wq, bq, Wk, Wv, Wg, ln_g, ln_b, Wz, Wo, ipc=IPC):
    import ml_dtypes
    del ln_b  # constant along j after softmax -> drops out exactly
    bf = ml_dtypes.bfloat16
    maps = []
    nsl = NCORES // B
    zbf = np.asarray(z, np.float32).astype(bf)
    Wbf = {n: np.asarray(w, np.float32).astype(bf)
           for n, w in (("Wq", Wq), ("Wk", Wk), ("Wv", Wv), ("Wg", Wg), ("Wo", Wo))}
    mask_is_ones = bool(np.all(np.asarray(mask) == 1.0))
    for c in range(NCORES):
        b, t = divmod(c, nsl)
        i0 = t * ipc
        m = {
            "zT": np.ascontiguousarray(
                zbf[b, i0:i0 + ipc].transpose(0, 2, 1)).reshape(ipc * CZ, N),
            "sT": np.ascontiguousarray(
                np.asarray(s, np.float32)[b, i0:i0 + ipc].T).astype(bf),
            "kinT": np.ascontiguousarray(
                np.asarray(k_in, np.float32)[b].T).astype(bf),
            "Wq": Wbf["Wq"], "Wk": Wbf["Wk"], "Wv": Wbf["Wv"],
            "Wg": Wbf["Wg"], "Wo": Wbf["Wo"],
            "Wz": np.ascontiguousarray(np.asarray(Wz, np.float32)),
            "ln_g": np.ascontiguousarray(
                np.asarray(ln_g, np.float32).reshape(CZ, 1)),
            "bqT": np.ascontiguousarray(
                np.asarray(bq, np.float32).reshape(CT_host, 128).T),
        }
        if not mask_is_ones:
            m["maskrow"] = np.ascontiguousarray(
                np.asarray(mask, np.float32)[b].reshape(1, N))
        maps.append(m)
    return maps


def kernel(**inputs):
    from concourse.bass_utils import run_bass_kernel_spmd
    mask_ones = bool(np.all(np.asarray(inputs["mask"]) == 1.0))
    nc = _get_prog(IPC, mask_ones)
    maps = _in_maps(**{k: np.asarray(v) for k, v in inputs.items()})
    trace = os.environ.get("KBENCH_TRACE", "") == "1"
    res = run_bass_kernel_spmd(nc, maps, core_ids=list(range(NCORES)), trace=trace)
    out = np.empty((B, N, CS), dtype=np.float32)
    nsl = NCORES // B
    for c in range(NCORES):
        b, t = divmod(c, nsl)
        out[b, t * IPC:(t + 1) * IPC, :] = res.results[c]["outT"].T
    if trace:
        print("HW exec time:", res.exec_time_ns, "ns")
    return out

